# revision 3
# baseline (speedup 1.0000x reference)
"""Triangular pairwise channel product on 8 Trainium2 NeuronCores.

out[b,h,w,k] = x[b,h,w,i_k] * x[b,h,w,j_k]  for the C*(C-1)/2 pairs
(i<j) in row-major (np.triu_indices) order.

Sharding: pure data parallel over batch — core c takes x[2c:2c+2].
Per core the 2*64*64 = 8192 spatial positions map to 128 SBUF
partitions (b_loc*64+h) x 64 groups (w).

The kernel is HBM-store bound: 66 MB of output per core at ~413 GB/s
of observed DMA ≈ 160 us of unavoidable store time.  fp32
tensor_tensor runs at 1 elem/cycle/lane on DVE plus ~151 cycles fixed
per instruction, so DVE alone (~190 us) cannot keep the store stream
fed.  The multiply work is split DVE (blocks i < I0) / GPSIMD (tail
blocks).  Two traps found by tracing:

* fp32 TT on DVE reads its second operand through the SBUF port pair
  that GPSIMD also uses — running both engines naively serializes them
  (~2.3 cyc/elem on both).  Fix: DVE's broadcast operand lives in PSUM
  (DVE has a separate PSUM read port); ACT, otherwise idle, stages each
  x chunk SBUF->PSUM.
* With whole-tile stores the two HWDGE rings never overlap and each
  iteration pays the full compute->store->release latency.  Fix: each
  iteration's output is two half tiles in separate pools (release at
  half granularity), stored on opposite rings as soon as their blocks
  finish.

Iteration sizes taper up at the start (stores begin early) and down at
the end (small final drain).
"""

import numpy as np

import concourse.bacc as bacc
import concourse.bass as bass
import concourse.mybir as mybir
import concourse.tile as tile
from concourse.bass_utils import run_bass_kernel_spmd

B, H, W, C = 16, 64, 64, 64
K = C * (C - 1) // 2  # 2016
N_CORES = 8
BP = B // N_CORES  # batch rows per core
P = BP * H         # 128 SBUF partitions
G_TOTAL = W        # position groups per partition
G_ITERS = [2, 5, 11, 11, 11, 11, 11, 2]
assert sum(G_ITERS) == W
G0 = G_ITERS[0]
GMAX = max(G_ITERS)
# Blocks i >= I0 (widths 63-I0 .. 1) run on GPSIMD, the rest on DVE.
I0 = 30
# Half-store boundary: blocks 0..MID-1 -> rows [0, row_MID) on ring A,
# the rest on ring B.  row_19 = 1024 ~ K/2.
MID = 19
FP = mybir.dt.float32

_row = [0]
for _i in range(C):
    _row.append(_row[-1] + C - 1 - _i)
RMID = _row[MID]

_nc_cache = None


def build_bass() -> bass.Bass:
    # Bacc (not plain Bass): its compile() pipeline runs
    # generate_event_semaphores, which splits multi-wait instructions to
    # satisfy the TRN2 1-wait-per-instruction codegen limit.
    nc = bacc.Bacc(
        "TRN2",
        target_bir_lowering=False,
        debug=False,
        num_devices=N_CORES,
    )
    x = nc.dram_tensor("x", [P, G_TOTAL, C], FP, kind="ExternalInput")
    y = nc.dram_tensor("y", [P, G_TOTAL, K], FP, kind="ExternalOutput")

    with tile.TileContext(nc) as tc:
        with (
            tc.tile_pool(name="xin", bufs=1) as xpool,
            tc.tile_pool(name="outA", bufs=2) as apool,
            tc.tile_pool(name="outB", bufs=2) as bpool,
            tc.tile_pool(name="xps", bufs=2, space="PSUM") as ppool,
        ):
            # Preload the input in two pieces: iteration 0's chunk on the
            # SP ring, the rest on the ACT ring.
            xt0 = xpool.tile([P, G0, C], FP, tag="x0")
            nc.sync.dma_start(out=xt0[:], in_=x[:, 0:G0, :])
            xtr = xpool.tile([P, G_TOTAL - G0, C], FP, tag="xr")
            nc.scalar.dma_start(out=xtr[:], in_=x[:, G0:, :])

            g_off = 0
            for it, Gi in enumerate(G_ITERS):
                if it == 0:
                    xg = xt0[:, :, :]
                else:
                    xg = xtr[:, g_off - G0 : g_off - G0 + Gi, :]

                # ACT stages the chunk into PSUM for DVE's broadcast
                # operands (keeps DVE off the GPSIMD-shared SBUF port).
                xp = ppool.tile([P, GMAX, C], FP, tag="xp")
                nc.scalar.copy(out=xp[:, 0:Gi, :], in_=xg)

                ota = apool.tile([P, Gi, RMID], FP, tag="ota")
                otb = bpool.tile([P, Gi, K - RMID], FP, tag="otb")

                def block(i):
                    w = C - 1 - i
                    a = xp[:, 0:Gi, i : i + 1].broadcast_to([P, Gi, w])
                    b = xg[:, :, i + 1 : C]
                    if _row[i] >= RMID:
                        o = otb[:, :, _row[i] - RMID : _row[i] - RMID + w]
                    else:
                        o = ota[:, :, _row[i] : _row[i] + w]
                    return a, b, o

                # GPSIMD tail blocks first so its queue starts immediately
                # (all-SBUF operands; shared port pair is free of DVE).
                for i in range(I0, C - 1):
                    a, b, o = block(i)
                    ag = xg[:, :, i : i + 1].broadcast_to([P, Gi, C - 1 - i])
                    nc.gpsimd.tensor_mul(o, ag, b)

                for i in range(0, MID):
                    a, b, o = block(i)
                    nc.vector.tensor_mul(o, a, b)
                nc.sync.dma_start(out=y[:, g_off : g_off + Gi, 0:RMID], in_=ota[:])

                for i in range(MID, I0):
                    a, b, o = block(i)
                    nc.vector.tensor_mul(o, a, b)
                nc.scalar.dma_start(
                    out=y[:, g_off : g_off + Gi, RMID:K], in_=otb[:]
                )
                g_off += Gi

    nc.finalize()
    return nc


def make_in_maps(x: np.ndarray) -> list[dict[str, np.ndarray]]:
    x = np.ascontiguousarray(x, dtype=np.float32)
    return [
        {"x": x[c * BP : (c + 1) * BP].reshape(P, G_TOTAL, C)} for c in range(N_CORES)
    ]


def kernel(**inputs: np.ndarray) -> np.ndarray:
    global _nc_cache
    if _nc_cache is None:
        _nc_cache = build_bass()
    res = run_bass_kernel_spmd(
        _nc_cache, make_in_maps(inputs["inputs"]), list(range(N_CORES))
    ).results
    return np.concatenate(
        [res[c]["y"].reshape(BP, H, W, K) for c in range(N_CORES)], axis=0
    )


# revision 4
# speedup vs baseline: 1.0600x; 1.0600x over previous
"""Triangular pairwise channel product on 8 Trainium2 NeuronCores.

out[b,h,w,k] = x[b,h,w,i_k] * x[b,h,w,j_k]  for the C*(C-1)/2 pairs
(i<j) in row-major (np.triu_indices) order.

Sharding: pure data parallel over batch — core c takes x[2c:2c+2].
Per core the 2*64*64 = 8192 spatial positions map to 128 SBUF
partitions (b_loc*64+h) x 64 groups (w).  Block i of the output (pairs
(i, i+1..63)) is one fp32 tensor_tensor multiply per group-chunk whose
first operand is x[..., i] broadcast via a step-0 access pattern.

The kernel is HBM-store bound: 66 MB of output per core at ~413 GB/s
observed DMA ≈ 160 us of unavoidable store time.  fp32 tensor_tensor
runs at 1 elem/cycle/lane + ~145 cyc/instr fixed on DVE, so DVE alone
(~190 us) cannot keep the store stream fed.  Design, driven by traces:

* Work split: blocks i < I0 on DVE, the small tail blocks on GPSIMD
  (~2.0 cyc/elem, otherwise idle).
* fp32 TT on DVE reads its second operand through the SBUF port pair
  GPSIMD uses — running both engines naively serializes them to
  ~2.3 cyc/elem.  Fix: DVE's broadcast operand lives in PSUM (separate
  DVE read port).  The PE — otherwise dead — stages each x chunk
  SBUF->PSUM via an identity matmul (I.T @ x), keeping the staging off
  every contended sequencer.
* Stores are monolithic [P, Gi, K] rows (contiguous per-partition DRAM
  runs, 128 descriptors; fragmented half stores cost 8.9 us of
  sequencer descriptor-gen and serialized the ACT ring).  Rings
  alternate per iteration; each ring sequencer only ever
  waits-for-compute and dispatches, nothing queues behind it.

Iteration sizes taper up at the start (stores begin early) and down at
the end (small final drain).
"""

import numpy as np

import concourse.bacc as bacc
import concourse.bass as bass
import concourse.mybir as mybir
import concourse.tile as tile
from concourse.bass_utils import run_bass_kernel_spmd

B, H, W, C = 16, 64, 64, 64
K = C * (C - 1) // 2  # 2016
N_CORES = 8
BP = B // N_CORES  # batch rows per core
P = BP * H         # 128 SBUF partitions
G_TOTAL = W        # position groups per partition
G_ITERS = [2, 5, 11, 11, 11, 11, 11, 2]
assert sum(G_ITERS) == W
G0 = G_ITERS[0]
GMAX = max(G_ITERS)
# Blocks i >= I0 (widths 63-I0 .. 1) run on GPSIMD, the rest on DVE.
I0 = 30
FP = mybir.dt.float32
MM_CHUNK = 512 // C  # groups per identity-matmul (moving free dim <= 512)

_row = [0]
for _i in range(C):
    _row.append(_row[-1] + C - 1 - _i)

_nc_cache = None


def build_bass() -> bass.Bass:
    # Bacc (not plain Bass): its compile() pipeline runs
    # generate_event_semaphores, which splits multi-wait instructions to
    # satisfy the TRN2 1-wait-per-instruction codegen limit.
    nc = bacc.Bacc(
        "TRN2",
        target_bir_lowering=False,
        debug=False,
        num_devices=N_CORES,
    )
    x = nc.dram_tensor("x", [P, G_TOTAL, C], FP, kind="ExternalInput")
    ident = nc.dram_tensor("ident", [P, P], FP, kind="ExternalInput")
    y = nc.dram_tensor("y", [P, G_TOTAL, K], FP, kind="ExternalOutput")

    with tile.TileContext(nc) as tc:
        with (
            tc.tile_pool(name="xin", bufs=1) as xpool,
            tc.tile_pool(name="out", bufs=2) as opool,
            tc.tile_pool(name="xps", bufs=2, space="PSUM") as ppool,
        ):
            # Preload: iteration 0's chunk + identity on the SP ring, the
            # remaining groups on the ACT ring.
            xt0 = xpool.tile([P, G0, C], FP, tag="x0")
            nc.sync.dma_start(out=xt0[:], in_=x[:, 0:G0, :])
            idt = xpool.tile([P, P], FP, tag="id")
            nc.sync.dma_start(out=idt[:], in_=ident[:, :])
            xtr = xpool.tile([P, G_TOTAL - G0, C], FP, tag="xr")
            nc.scalar.dma_start(out=xtr[:], in_=x[:, G0:, :])

            g_off = 0
            for it, Gi in enumerate(G_ITERS):
                if it == 0:
                    xg = xt0[:, :, :]
                else:
                    xg = xtr[:, g_off - G0 : g_off - G0 + Gi, :]

                # PE stages the chunk into PSUM (xp = I.T @ xg) for DVE's
                # broadcast operands, keeping DVE off the GPSIMD-shared
                # SBUF port and the staging off the DMA sequencers.
                xp = ppool.tile([P, GMAX, C], FP, tag="xp")
                for c0 in range(0, Gi, MM_CHUNK):
                    c1 = min(c0 + MM_CHUNK, Gi)
                    nc.tensor.matmul(
                        out=xp[:, c0:c1, :], lhsT=idt[:], rhs=xg[:, c0:c1, :]
                    )

                ot = opool.tile([P, Gi, K], FP, tag="ot")

                # GPSIMD tail blocks first so its queue starts immediately
                # (all-SBUF operands; DVE never touches the shared pair).
                for i in range(I0, C - 1):
                    w = C - 1 - i
                    a = xg[:, :, i : i + 1].broadcast_to([P, Gi, w])
                    b = xg[:, :, i + 1 : C]
                    nc.gpsimd.tensor_mul(ot[:, :, _row[i] : _row[i] + w], a, b)

                for i in range(0, I0):
                    w = C - 1 - i
                    a = xp[:, 0:Gi, i : i + 1].broadcast_to([P, Gi, w])
                    b = xg[:, :, i + 1 : C]
                    nc.vector.tensor_mul(ot[:, :, _row[i] : _row[i] + w], a, b)

                # Full 2016-channel rows -> contiguous per-partition DRAM
                # runs; alternate HWDGE rings.
                ring = nc.sync if it % 2 == 0 else nc.scalar
                ring.dma_start(out=y[:, g_off : g_off + Gi, :], in_=ot[:])
                g_off += Gi

    nc.finalize()
    return nc


def make_in_maps(x: np.ndarray) -> list[dict[str, np.ndarray]]:
    x = np.ascontiguousarray(x, dtype=np.float32)
    eye = np.eye(P, dtype=np.float32)
    return [
        {"x": x[c * BP : (c + 1) * BP].reshape(P, G_TOTAL, C), "ident": eye}
        for c in range(N_CORES)
    ]


def kernel(**inputs: np.ndarray) -> np.ndarray:
    global _nc_cache
    if _nc_cache is None:
        _nc_cache = build_bass()
    res = run_bass_kernel_spmd(
        _nc_cache, make_in_maps(inputs["inputs"]), list(range(N_CORES))
    ).results
    return np.concatenate(
        [res[c]["y"].reshape(BP, H, W, K) for c in range(N_CORES)], axis=0
    )
